# revision 1
# baseline (speedup 1.0000x reference)
"""Trainium2 Bass kernel for nn_DE3 (histogram_binning + entropy).

Full input: img [16, 2048, 2048] f32 with values in [0, 256).
reference = B * (8 - res), res = -sum p log2 p, p = bincount(floor(img)) / (H*W).

Strategy (8 NeuronCores, data parallel):
  - Split the 64Mi elements into 8 shards of 8Mi (one per core).
  - Per core, compute the 2-D cumulative-count matrix
        J[i, j] = #{e : hi_e >= i AND lo_e >= j}, i in [0,NHI), j in [0,NLO)
    where idx = floor(x) = NLO*hi + lo (NHI x NLO = 256 bins). J is
    accumulated on the PE (one [128,NHI]x[128,NLO] bf16 matmul per 128
    elements into a single PSUM tile; NHI=8 keeps the LDWEIGHTS stream
    short). The hi/lo "ladders" (is_ge cumulants) are built on DVE/ACT
    at a few cycles per element via the float32 round-to-int trick.
  - Host: sum J over cores, 2-D finite difference -> 256-bin counts,
    then the trivial entropy epilogue.
"""

import numpy as np

import concourse.bass as bass
import concourse.mybir as mybir
from concourse.tile import TileContext
from concourse.bass_utils import run_bass_kernel_spmd

P = 128          # SBUF partitions
F = 512          # free-dim elements per tile
N_CORES = 8
NHI = 16         # hi (coarse) bins  -> matmul M / LDWEIGHTS columns
NLO = 16         # lo (fine) bins within a block -> matmul N
assert NHI * NLO == 256

_BIG = float(3 * 2**22)  # 1.5*2^23: keeps t in [2^23, 2^24) where ulp = 1

_MAX_WAITS = 1  # this walrus build supports at most 1 sync-wait per instruction


def _split_excess_waits(nc):
    """Walrus in this container rejects instructions with >2 sync-wait
    commands (Tile's tail drain can carry many). Move excess waits onto
    same-engine NoOp instructions inserted just before the offender."""
    n_split = 0
    for f in nc.m.functions:
        for bb in f.blocks:
            out = []
            for ins in bb.instructions:
                si = getattr(ins, "sync_info", None)
                waits = list(si.on_wait) if si is not None and si.on_wait else []
                if len(waits) > _MAX_WAITS:
                    extra, keep = waits[:-_MAX_WAITS], waits[-_MAX_WAITS:]
                    for ci in range(0, len(extra), _MAX_WAITS):
                        chunk = extra[ci : ci + _MAX_WAITS]
                        nop = mybir.InstNoOp(
                            name=f"{ins.name}-wsplit{ci}",
                            engine=ins.engine,
                            sync_info=mybir.SyncInfo(on_wait=chunk, on_update=[]),
                        )
                        out.append(nop)
                        n_split += 1
                    si.on_wait = keep
                out.append(ins)
            bb.instructions = out
    return n_split


def build_nc(n_tiles: int, debug: bool = False, repeat: int = 1, col_tiles: int = 1):
    """Build the Bass kernel: input [n_tiles*P, F] f32 -> output J [16,16] f32."""
    nc = bass.Bass()
    # const AP for the ACT-engine bias (-2^23), mirroring Bass's own init
    _ct = nc.alloc_sbuf_tensor("const-neg-big", [128, 1], mybir.dt.float32)
    nc.gpsimd.memset(_ct.ap(), -_BIG)
    nc.const_aps.aps[(mybir.dt.float32, -_BIG)] = _ct.ap()
    nc.all_engine_barrier()
    x_in = nc.declare_dram_parameter(
        "x", [n_tiles * P, F], mybir.dt.float32, isOutput=False
    )
    j_out = nc.declare_dram_parameter(
        "j", [col_tiles * NHI, NLO], mybir.dt.float32, isOutput=True
    )
    if debug:
        dbg_hi = nc.declare_dram_parameter("dbg_hi", [P, F], mybir.dt.float32, isOutput=True)
        dbg_lo = nc.declare_dram_parameter("dbg_lo", [P, F], mybir.dt.float32, isOutput=True)
        dbg_lhi = nc.declare_dram_parameter("dbg_lhi", [P, NHI * F], mybir.dt.float32, isOutput=True)
        dbg_llo = nc.declare_dram_parameter("dbg_llo", [P, NLO * F], mybir.dt.float32, isOutput=True)

    dt = mybir.dt
    op = mybir.AluOpType

    with TileContext(nc) as tc:
        with (
            tc.tile_pool(name="data", bufs=3) as dpool,
            tc.tile_pool(name="lad", bufs=2) as lpool,
            tc.tile_pool(name="psum", bufs=1, space="PSUM") as ppool,
            tc.tile_pool(name="outp", bufs=1) as opool,
        ):
            # col_tiles > 1: spread chunks round-robin over PE column
            # groups; each group accumulates its own J slice at PSUM
            # partition base 32*g (summed on the host afterwards).
            jt = ppool.tile([32 * (col_tiles - 1) + NHI, NLO], dt.float32)
            for rep in range(repeat):
              for it in range(n_tiles):
                  x = dpool.tile([P, F], dt.float32, tag="x")
                  nc.sync.dma_start(out=x[:], in_=x_in[it * P : (it + 1) * P, :])
                  # xb = x - NLO/2 (exact); carries the -0.5 through /NLO for
                  # the floor-by-round trick (BIG-0.5 is not representable).
                  xb = dpool.tile([P, F], dt.float32, tag="xb")
                  nc.vector.tensor_scalar(
                      out=xb[:], in0=x[:], scalar1=-float(NLO) / 2.0, scalar2=None, op0=op.add
                  )
                  # t = xb/NLO + BIG = (x/NLO - 0.5) + BIG -> RN: BIG + floor(x/NLO)
                  t = dpool.tile([P, F], dt.float32, tag="t")
                  nc.vector.tensor_scalar(
                      out=t[:], in0=xb[:], scalar1=1.0 / float(NLO), scalar2=_BIG,
                      op0=op.mult, op1=op.add,
                  )
                  # hi = t - BIG in [0,NHI], exact small int -> bf16 (ACT engine)
                  hi8 = dpool.tile([P, F], dt.bfloat16, tag="hi8")
                  nc.scalar.add(hi8[:], t[:], -_BIG)
                  # yb = xb - NLO*hi = (x - NLO*hi) - NLO/2  in [-NLO/2, NLO/2)
                  yb = dpool.tile([P, F], dt.float32, tag="yb")
                  nc.vector.scalar_tensor_tensor(
                      out=yb[:], in0=hi8[:], scalar=-float(NLO), in1=xb[:],
                      op0=op.mult, op1=op.add,
                  )
                  # u = (yb + (NLO/2 - 0.5)) + BIG -> RN: BIG + lo
                  u = dpool.tile([P, F], dt.float32, tag="u")
                  nc.vector.tensor_scalar(
                      out=u[:], in0=yb[:], scalar1=float(NLO) / 2.0 - 0.5, scalar2=_BIG,
                      op0=op.add, op1=op.add,
                  )
                  # lo = u - BIG in [0,NLO], exact small int -> bf16 (ACT engine)
                  lo8 = dpool.tile([P, F], dt.bfloat16, tag="lo8")
                  nc.scalar.add(lo8[:], u[:], -_BIG)

                  # ladders: lhi[p, i, f] = (hi >= i), llo[p, j, f] = (lo >= j)
                  lhi = lpool.tile([P, NHI, F], dt.bfloat16, tag="lhi")
                  llo = lpool.tile([P, NLO, F], dt.bfloat16, tag="llo")
                  for j in range(NHI):
                      nc.vector.tensor_scalar(
                          out=lhi[:, j, :], in0=hi8[:], scalar1=float(j), scalar2=None,
                          op0=op.is_ge,
                      )
                  for j in range(NLO):
                      nc.vector.tensor_scalar(
                          out=llo[:, j, :], in0=lo8[:], scalar1=float(j), scalar2=None,
                          op0=op.is_ge,
                      )
                  if debug and it == 0:
                      fhi = dpool.tile([P, F], dt.float32, tag="fhi")
                      nc.vector.tensor_copy(out=fhi[:], in_=hi8[:])
                      nc.sync.dma_start(out=dbg_hi[:], in_=fhi[:])
                      flo = dpool.tile([P, F], dt.float32, tag="flo")
                      nc.vector.tensor_copy(out=flo[:], in_=lo8[:])
                      nc.sync.dma_start(out=dbg_lo[:], in_=flo[:])
                      flh = lpool.tile([P, NHI * F], dt.float32, tag="flh")
                      nc.vector.tensor_copy(out=flh[:], in_=lhi[:].rearrange('p a b -> p (a b)'))
                      nc.sync.dma_start(out=dbg_lhi[:], in_=flh[:])
                      fll = lpool.tile([P, NLO * F], dt.float32, tag="fll")
                      nc.vector.tensor_copy(out=fll[:], in_=llo[:].rearrange('p a b -> p (a b)'))
                      nc.sync.dma_start(out=dbg_llo[:], in_=fll[:])
                  # PE: accumulate J += lhi_c^T @ llo_c for each 128-elem column c
                  for c in range(F):
                      g = c % col_tiles
                      nc.tensor.matmul(
                          jt[32 * g : 32 * g + NHI, :],
                          lhsT=lhi[:, :, c],
                          rhs=llo[:, :, c],
                          start=(rep == 0 and it == 0 and c < col_tiles),
                          stop=(rep == repeat - 1 and it == n_tiles - 1 and c >= F - col_tiles),
                          tile_position=(0, 32 * g) if col_tiles > 1 else None,
                      )
            jsb = opool.tile([32 * (col_tiles - 1) + NHI, NLO], dt.float32)
            for g in range(col_tiles):
                nc.vector.tensor_copy(
                    out=jsb[32 * g : 32 * g + NHI, :],
                    in_=jt[32 * g : 32 * g + NHI, :],
                )
                nc.sync.dma_start(
                    out=j_out[g * NHI : (g + 1) * NHI, :],
                    in_=jsb[32 * g : 32 * g + NHI, :],
                )
    _split_excess_waits(nc)
    return nc


def _counts_from_J(J: np.ndarray) -> np.ndarray:
    """J [NHI,NLO] cumulative -> counts [256] (bin = NLO*hi + lo)."""
    Jp = np.zeros((NHI + 1, NLO + 1), dtype=np.float64)
    Jp[:NHI, :NLO] = J
    A = Jp[:NHI, :] - Jp[1:, :]
    c2 = A[:, :NLO] - A[:, 1:]
    return c2.reshape(256)


def kernel(img: np.ndarray) -> np.ndarray:
    img = np.asarray(img, dtype=np.float32)
    B, H, W = img.shape
    flat = img.reshape(-1)
    n = flat.size
    assert n % (N_CORES * P * F) == 0
    shard = n // N_CORES
    n_tiles = shard // (P * F)

    nc = build_nc(n_tiles)
    in_maps = [
        {"x": flat[i * shard : (i + 1) * shard].reshape(n_tiles * P, F)}
        for i in range(N_CORES)
    ]
    res = run_bass_kernel_spmd(nc, in_maps, list(range(N_CORES)))
    J = np.zeros((NHI, NLO), dtype=np.float64)
    for r in res.results:
        J += np.asarray(r["j"], dtype=np.float64)

    counts = _counts_from_J(J)
    temp = float(H * W)
    p = counts / temp
    with np.errstate(divide="ignore", invalid="ignore"):
        terms = np.where(p > 0, p * np.log2(np.where(p > 0, p, 1.0)), 0.0)
    ent = -terms.sum()
    out = np.float32(B * (8.0 - ent))
    return np.asarray(out, dtype=np.float32)



# revision 2
# speedup vs baseline: 2.2682x; 2.2682x over previous
"""Trainium2 Bass kernel v2 for nn_DE3 (histogram_binning + entropy).

Full input: img [16, 2048, 2048] f32, values in [0, 256).
reference = B * (8 - res), res = -sum p log2 p, p = bincount(floor(img)) / (H*W).

Strategy (8 NeuronCores, data parallel, 8.4M elements/core):
  - Two-level radix: bin = 16*hi + lo, hi = floor(x/16), lo = floor(x) - 16*hi.
    Exact fp32 floor via the round-to-int trick t + 2^23 with a +2^-20
    tie guard (inputs are multiples of 2^-15 -> provably exact, no ties).
  - Per element, "ladder" indicators lhi_i = (hi >= i), llo_j = (lo >= j)
    (one tensor_scalar is_ge per level, bf16 -> DVE 4x mode; some levels
    on ACT as +-1 Sign activations, unmixed exactly on the host; some on
    GPSIMD as tensor_tensor is_ge vs a constant tile).
  - PE: pack CH=8 chunks of 128 elements into one [128,128]x[128,128]
    bf16 matmul (lhsT col 8i+a = hi-level i of chunk a, rhs col 8j+b =
    lo-level j of chunk b) accumulating a [128,128] PSUM; the 8 diagonal
    16x16 blocks hold J[i,j] = #{hi>=i and lo>=j} contributions.
  - On-device diagonal-block extraction via 8 tiny f32 matmuls against
    identity slices -> DRAM output is just [16,16] f32 per core.
  - Host: sum over cores, unmix Sign levels, 2-D finite difference ->
    256 exact counts, entropy epilogue in float64.
"""

import numpy as np

import concourse.bass as bass
import concourse.mybir as mybir
from concourse.tile import TileContext
from concourse.bass_utils import run_bass_kernel_spmd

P = 128           # SBUF partitions
F = 1024          # free-dim elements per tile
CH = 8            # chunks (SBUF columns) packed per matmul
N_CORES = 8
NHI = 16          # hi (coarse) bins
NLO = 16          # lo (fine) bins
assert NHI * NLO == 256

_BIG = float(3 * 2**22)   # 1.5*2^23: t in [2^23, 2^24) where ulp = 1
_EPS = float(2.0 ** -20)  # tie-breaking guard (inputs are multiples of 2^-15)
_C1 = -0.5 + _EPS         # added to x/16 before the BIG round -> floor
_C2 = -0.5 + _EPS         # added to r before the BIG round -> floor

_MAX_WAITS = 1  # this walrus build supports at most 1 sync-wait per instruction

# ---- engine-placement tuning knobs ----
ACT_HI_LEVELS: tuple = (13, 14, 15)   # +-1 Sign levels on ACT (host-unmixed)
ACT_LO_LEVELS: tuple = (13, 14, 15)
GP_HI_LEVELS: tuple = ()                  # 0/1 is_ge levels on GPSIMD
GP_LO_LEVELS: tuple = ()
T_ON_ACT = True       # t = Identity(x/16 + c1) on ACT
HI8_ON_ACT = False    # hi8 via two ACT adds (else fused DVE tensor_scalar)
U_ON_ACT = True       # u via two ACT adds (else fused DVE tensor_scalar)
MEMSET_J0 = False     # j=0 ladder planes (const 1) via GPSIMD memset
DATA_BUFS = 3
LAD_BUFS = 2
ONLY_FIRST_MM = False       # diagnostic: matmuls only on tile 0
ONLY_FIRST_LADDERS = False  # diagnostic: ladders/prep only on tile 0
LAD_DT = "bfloat16"


def _split_excess_waits(nc):
    """Walrus in this container rejects instructions with >2 sync-wait
    commands (Tile's tail drain can carry many). Move excess waits onto
    same-engine NoOp instructions inserted just before the offender."""
    n_split = 0
    for f in nc.m.functions:
        for bb in f.blocks:
            out = []
            for ins in bb.instructions:
                si = getattr(ins, "sync_info", None)
                waits = list(si.on_wait) if si is not None and si.on_wait else []
                if len(waits) > _MAX_WAITS:
                    extra, keep = waits[:-_MAX_WAITS], waits[-_MAX_WAITS:]
                    for ci in range(0, len(extra), _MAX_WAITS):
                        chunk = extra[ci : ci + _MAX_WAITS]
                        nop = mybir.InstNoOp(
                            name=f"{ins.name}-wsplit{ci}",
                            engine=ins.engine,
                            sync_info=mybir.SyncInfo(on_wait=chunk, on_update=[]),
                        )
                        out.append(nop)
                        n_split += 1
                    si.on_wait = keep
                out.append(ins)
            bb.instructions = out
    return n_split


def build_nc(n_tiles: int, repeat: int = 1):
    """Input x [n_tiles*P, F] f32 -> output j16 [16,16] f32 (+ident [128,128])."""
    nc = bass.Bass()
    # const APs for ACT-engine biases, mirroring Bass's own init
    consts = {-_BIG, float(_BIG)}
    consts.update(0.5 - j for j in ACT_HI_LEVELS)
    consts.update(0.5 - j for j in ACT_LO_LEVELS)
    if T_ON_ACT:
        consts.add(_C1)
    if U_ON_ACT:
        consts.add(_C2)
    for ci, cv in enumerate(sorted(consts)):
        _ct = nc.alloc_sbuf_tensor(f"const-c{ci}", [128, 1], mybir.dt.float32)
        nc.gpsimd.memset(_ct.ap(), cv)
        nc.const_aps.aps[(mybir.dt.float32, cv)] = _ct.ap()
    nc.all_engine_barrier()
    x_in = nc.declare_dram_parameter(
        "x", [n_tiles * P, F], mybir.dt.float32, isOutput=False
    )
    id_in = nc.declare_dram_parameter(
        "ident", [P, P], mybir.dt.float32, isOutput=False
    )
    j_out = nc.declare_dram_parameter("j", [NHI, NLO], mybir.dt.float32, isOutput=True)

    dt = mybir.dt
    op = mybir.AluOpType
    lad_dt = getattr(dt, LAD_DT)
    n_groups = F // CH
    act = mybir.ActivationFunctionType

    with TileContext(nc) as tc:
        with (
            tc.tile_pool(name="data", bufs=DATA_BUFS) as dpool,
            tc.tile_pool(name="lad", bufs=LAD_BUFS) as lpool,
            tc.tile_pool(name="psum", bufs=1, space="PSUM") as ppool,
            tc.tile_pool(name="outp", bufs=1) as opool,
        ):
            ident = opool.tile([P, P], dt.float32)
            nc.sync.dma_start(out=ident[:], in_=id_in[:, :])

            # constant threshold tiles for GPSIMD is_ge levels
            gp_consts = {}
            for j in sorted(set(GP_HI_LEVELS) | set(GP_LO_LEVELS)):
                cj = opool.tile([P, F], lad_dt)
                nc.gpsimd.memset(cj[:], float(j))
                gp_consts[j] = cj

            lad_bufs = []
            if ONLY_FIRST_LADDERS:
                lhi_b = opool.tile([P, n_groups, NHI, CH], lad_dt)
                llo_b = opool.tile([P, n_groups, NLO, CH], lad_dt)
                lad_bufs.append((lhi_b, llo_b))

            jt = ppool.tile([P, P], dt.float32)
            mm_last = 0 if ONLY_FIRST_MM else n_tiles - 1
            for rep in range(repeat):
                for it in range(n_tiles):
                    x = dpool.tile([P, F], dt.float32, tag="x")
                    nc.sync.dma_start(out=x[:], in_=x_in[it * P : (it + 1) * P, :])
                    do_ladders = (it == 0 and rep == 0) or not ONLY_FIRST_LADDERS
                    do_mm = it == 0 or not ONLY_FIRST_MM
                    if not do_ladders:
                        lhi, llo = lad_bufs[0]
                        if do_mm:
                            for g in range(n_groups):
                                nc.tensor.matmul(
                                    jt[:, :], lhsT=lhi[:, g, :, :], rhs=llo[:, g, :, :],
                                    start=False,
                                    stop=(rep == repeat - 1 and it == n_tiles - 1
                                          and g == n_groups - 1),
                                )
                        continue
                    # t = x/16 + (-0.5 + eps)   (exact fp32 for x = k*2^-15)
                    t = dpool.tile([P, F], dt.float32, tag="t")
                    if T_ON_ACT:
                        nc.scalar.activation(
                            t[:], x[:], act.Identity, bias=_C1, scale=1.0 / 16.0
                        )
                    else:
                        nc.vector.tensor_scalar(
                            out=t[:], in0=x[:], scalar1=1.0 / 16.0, scalar2=_C1,
                            op0=op.mult, op1=op.add,
                        )
                    # hi = RN(t + BIG) - BIG = floor(x/16), exact -> bf16
                    hi8 = dpool.tile([P, F], dt.bfloat16, tag="hi8")
                    if HI8_ON_ACT:
                        a1 = dpool.tile([P, F], dt.float32, tag="a1")
                        nc.scalar.add(a1[:], t[:], _BIG)
                        nc.scalar.add(hi8[:], a1[:], -_BIG)
                    else:
                        nc.vector.tensor_scalar(
                            out=hi8[:], in0=t[:], scalar1=_BIG, scalar2=-_BIG,
                            op0=op.add, op1=op.add,
                        )
                    # r = x - 16*hi in [0,16)
                    r = dpool.tile([P, F], dt.float32, tag="r")
                    nc.vector.scalar_tensor_tensor(
                        out=r[:], in0=hi8[:], scalar=-16.0, in1=x[:],
                        op0=op.mult, op1=op.add,
                    )
                    # u = (r + c2) + BIG -> BIG + floor(r)
                    u = dpool.tile([P, F], dt.float32, tag="u")
                    if U_ON_ACT:
                        u1 = dpool.tile([P, F], dt.float32, tag="u1")
                        nc.scalar.add(u1[:], r[:], _C2)
                        nc.scalar.add(u[:], u1[:], _BIG)
                    else:
                        nc.vector.tensor_scalar(
                            out=u[:], in0=r[:], scalar1=_C2, scalar2=_BIG,
                            op0=op.add, op1=op.add,
                        )
                    # lo = u - BIG, exact small int -> bf16 (ACT engine)
                    lo8 = dpool.tile([P, F], dt.bfloat16, tag="lo8")
                    nc.scalar.add(lo8[:], u[:], -_BIG)

                    # ladders, group-major layout: lhi[p, g, i, a] = (hi[p, CH*g+a] >= i)
                    # so matmul operand lhi[:, g, :, :] is one contiguous free dim.
                    lhi = lpool.tile([P, n_groups, NHI, CH], lad_dt, tag="lhi")
                    llo = lpool.tile([P, n_groups, NLO, CH], lad_dt, tag="llo")
                    if ONLY_FIRST_LADDERS:
                        lhi, llo = lad_bufs[0]
                    hi8v = hi8[:].rearrange("p (g a) -> p g a", a=CH)
                    lo8v = lo8[:].rearrange("p (g a) -> p g a", a=CH)
                    for lad, src, act_set, gp_set, n_lev in (
                        (lhi, hi8v, ACT_HI_LEVELS, GP_HI_LEVELS, NHI),
                        (llo, lo8v, ACT_LO_LEVELS, GP_LO_LEVELS, NLO),
                    ):
                        for j in range(n_lev):
                            if j == 0 and MEMSET_J0:
                                nc.gpsimd.memset(lad[:, :, 0, :], 1.0)
                            elif j in act_set:
                                nc.scalar.activation(
                                    lad[:, :, j, :], src, act.Sign,
                                    bias=0.5 - j, scale=1.0,
                                )
                            elif j in gp_set:
                                nc.gpsimd.tensor_tensor(
                                    out=lad[:, :, j, :], in0=src,
                                    in1=gp_consts[j][:].rearrange(
                                        "p (g a) -> p g a", a=CH),
                                    op=op.is_ge,
                                )
                            else:
                                nc.vector.tensor_scalar(
                                    out=lad[:, :, j, :], in0=src, scalar1=float(j),
                                    scalar2=None, op0=op.is_ge,
                                )

                    # PE: one [128,128]x[128,128] matmul per CH-chunk group
                    if do_mm:
                        for g in range(n_groups):
                            nc.tensor.matmul(
                                jt[:, :],
                                lhsT=lhi[:, g, :, :],
                                rhs=llo[:, g, :, :],
                                start=(rep == 0 and it == 0 and g == 0),
                                stop=(rep == repeat - 1 and it == mm_last
                                      and g == n_groups - 1),
                            )

            # diagonal-block extraction: J16[i,j] = sum_a P[8i+a, 8j+a]
            psb = opool.tile([P, P], dt.float32)
            nc.vector.tensor_copy(out=psb[:], in_=jt[:, :])
            j16 = ppool.tile([NHI, NLO], dt.float32)
            for a in range(CH):
                nc.tensor.matmul(
                    j16[:, :],
                    lhsT=ident[:, a :: CH],
                    rhs=psb[:, a :: CH],
                    start=(a == 0),
                    stop=(a == CH - 1),
                )
            j16sb = opool.tile([NHI, NLO], dt.float32)
            nc.vector.tensor_copy(out=j16sb[:], in_=j16[:, :])
            nc.sync.dma_start(out=j_out[:, :], in_=j16sb[:])

    _split_excess_waits(nc)
    return nc


def _ladder_mix_matrix(n, sign_levels):
    """A with f = A @ l, where l = step ladders (l_0 == 1): step rows e_i,
    sign rows 2 e_i - e_0."""
    A = np.eye(n, dtype=np.float64)
    for j in sign_levels:
        A[j, j] = 2.0
        A[j, 0] -= 1.0
    return A


def _counts_from_J(J: np.ndarray) -> np.ndarray:
    """J [NHI,NLO] cumulative -> counts [256] (bin = NLO*hi + lo)."""
    Jp = np.zeros((NHI + 1, NLO + 1), dtype=np.float64)
    Jp[:NHI, :NLO] = J
    A = Jp[:NHI, :] - Jp[1:, :]
    c2 = A[:, :NLO] - A[:, 1:]
    return c2.reshape(256)


def _ident_for_extract() -> np.ndarray:
    return np.eye(P, dtype=np.float32)


def _J_from_results(results) -> np.ndarray:
    S = np.zeros((NHI, NLO), dtype=np.float64)
    for r in results:
        S += np.asarray(r["j"], dtype=np.float64)
    if ACT_HI_LEVELS or ACT_LO_LEVELS:
        Ahi = _ladder_mix_matrix(NHI, ACT_HI_LEVELS)
        Alo = _ladder_mix_matrix(NLO, ACT_LO_LEVELS)
        S = np.linalg.solve(Ahi, S)
        S = np.linalg.solve(Alo, S.T).T
        S = np.round(S)
    return S


def kernel(img: np.ndarray) -> np.ndarray:
    img = np.asarray(img, dtype=np.float32)
    B, H, W = img.shape
    flat = img.reshape(-1)
    n = flat.size
    assert n % (N_CORES * P * F) == 0
    shard = n // N_CORES
    n_tiles = shard // (P * F)

    nc = build_nc(n_tiles)
    ident = _ident_for_extract()
    in_maps = [
        {"x": flat[i * shard : (i + 1) * shard].reshape(n_tiles * P, F),
         "ident": ident}
        for i in range(N_CORES)
    ]
    res = run_bass_kernel_spmd(nc, in_maps, list(range(N_CORES)))
    J = _J_from_results(res.results)

    counts = _counts_from_J(J)
    temp = float(H * W)
    p = counts / temp
    with np.errstate(divide="ignore", invalid="ignore"):
        terms = np.where(p > 0, p * np.log2(np.where(p > 0, p, 1.0)), 0.0)
    ent = -terms.sum()
    out = np.float32(B * (8.0 - ent))
    return np.asarray(out, dtype=np.float32)
